# revision 1
# baseline (speedup 1.0000x reference)
"""Llama attention layer on 8 TRN2 NeuronCores.

Sharding: core = (batch b in 0..1) x (head-group g in 0..3), 4 heads each.
Per core: full hidden_states[b] (transposed on host), column slices of
wq/wk/wv, row slice of wo.T. Host sums the 4 per-head-group o_proj
partials per batch.

Layout strategy (per core):
  xT [H, S]      hidden.T     -> SBUF bf16, 16 k-chunk tiles [128, S]
  qT/kT [d, S]   per head     -> projections computed directly in
                                 transposed form (lhsT=wT chunk,
                                 rhs=xT chunk), RoPE applied from PSUM.
  v  [S, hg]     natural      -> lhsT=xT chunk, rhs=wvT chunk.
  scoresT [s_k, s_q] chunks   -> lhsT=kT tile, rhs=qT chunk; exp via
                                 ACT with per-partition mask bias
                                 (softmax w/o max subtraction: scores
                                 are ~N(0,1), exp is safe in f32/bf16).
  attn_outT [d, s_q]          -> lhsT=v tile, rhs=expT; denominator via
                                 ones-vector matmul; normalize with
                                 reciprocal + K=1 broadcast matmul.
  out [S, H] partial          -> lhsT=attn_outT slice, rhs=woT tile.
"""

import numpy as np
import ml_dtypes

B, S, H, NH, HD = 2, 2048, 2048, 16, 128
G = 4            # heads per core
HG = G * HD      # 512 head-dim columns per core
KT = H // 128    # 16 contraction chunks
ST = S // 128    # 16 sequence tiles of 128
SC = S // 512    # 4 sequence chunks of 512
NCORES = 8

_NC_CACHE = {}


def _ensure_path():
    import sys
    for p in ('/opt/trn_rl_repo', '/opt/pypackages'):
        if p not in sys.path:
            sys.path.append(p)


def _build_nc():
    import os
    PHASES = os.environ.get('KPHASES', '123')
    _ensure_path()
    from contextlib import ExitStack
    import concourse.tile as tile
    from concourse import bacc, mybir

    bf16 = mybir.dt.bfloat16
    f32 = mybir.dt.float32
    EXP = mybir.ActivationFunctionType.Exp

    nc = bacc.Bacc('TRN2', target_bir_lowering=False, debug=False)

    xT = nc.dram_tensor('xT', [H, S], bf16, kind='ExternalInput')
    wqT = nc.dram_tensor('wqT', [H, HG], bf16, kind='ExternalInput')
    wkT = nc.dram_tensor('wkT', [H, HG], bf16, kind='ExternalInput')
    wvT = nc.dram_tensor('wvT', [H, HG], bf16, kind='ExternalInput')
    woT = nc.dram_tensor('woT', [HG, H], bf16, kind='ExternalInput')
    cosT = nc.dram_tensor('cosT', [HD, S], f32, kind='ExternalInput')
    s2T = nc.dram_tensor('s2T', [HD, S], f32, kind='ExternalInput')
    maskb = nc.dram_tensor('maskb', [128, ST], f32, kind='ExternalInput')
    out = nc.dram_tensor('out', [S, H], f32, kind='ExternalOutput')

    with tile.TileContext(nc) as tc, ExitStack() as top:
        persist = top.enter_context(tc.tile_pool(name='persist', bufs=1))

        cos_t = persist.tile([HD, S], f32, tag='cos', name='cos')
        nc.sync.dma_start(cos_t[:], cosT[:])
        s2_t = persist.tile([HD, S], f32, tag='s2', name='s2')
        nc.sync.dma_start(s2_t[:], s2T[:])
        mb_t = persist.tile([128, ST], f32, tag='mb', name='mb')
        nc.sync.dma_start(mb_t[:], maskb[:])
        ones_t = persist.tile([128, 1], bf16, tag='ones', name='ones')
        nc.vector.memset(ones_t[:], 1.0)
        onesb_t = persist.tile([1, 128], f32, tag='onesb', name='onesb')
        nc.vector.memset(onesb_t[:], 1.0)

        # persistent activations
        qk = {
            nm: [persist.tile([128, S], bf16, tag=f'{nm}{i}', name=f'{nm}{i}') for i in range(G)]
            for nm in ('q', 'k')
        }
        vt = [persist.tile([128, HG], bf16, tag=f'v{i}', name=f'v{i}') for i in range(ST)]

        # ---------------- Phase 1: QKV projections + RoPE ----------------
        with tc.tile_pool(name='xw', bufs=1) as xw, \
             tc.tile_pool(name='pp', bufs=int(os.environ.get('B_PP','4')), space='PSUM') as pp, \
             tc.tile_pool(name='stg', bufs=int(os.environ.get('B_STG','4'))) as stg:
            xt = []
            for k in range(KT):
                t = xw.tile([128, S], bf16, tag=f'x{k}', name=f'x{k}')
                nc.sync.dma_start(t[:], xT[k * 128:(k + 1) * 128, :])
                xt.append(t)
            wts = {}
            for nm, dram in (('q', wqT), ('k', wkT), ('v', wvT)):
                wts[nm] = []
                for k in range(KT):
                    t = xw.tile([128, HG], bf16, tag=f'w{nm}{k}', name=f'w{nm}{k}')
                    nc.sync.dma_start(t[:], dram[k * 128:(k + 1) * 128, :])
                    wts[nm].append(t)

            # qT / kT: out tile [d=128 (head i), s chunk 512], then RoPE
            for nm in ('q', 'k'):
                for i in range(G):
                    dst = qk[nm][i]
                    for c in range(SC):
                        cs = slice(c * 512, (c + 1) * 512)
                        ps = pp.tile([128, 512], f32, tag='pp', name='pp')
                        for k in range(KT):
                            nc.tensor.matmul(
                                ps[:],
                                lhsT=wts[nm][k][:, i * 128:(i + 1) * 128],
                                rhs=xt[k][:, cs],
                                start=(k == 0), stop=(k == KT - 1),
                            )
                        # rope: out[d] = x[d]*cos[d] + x[(d+64)%128]*s2[d]
                        t1 = stg.tile([128, 512], f32, tag='t1', name='t1')
                        nc.vector.tensor_mul(t1[:], ps[:], cos_t[:, cs])
                        t2 = stg.tile([128, 512], f32, tag='t2', name='t2')
                        nc.vector.tensor_mul(t2[0:64, :], ps[64:128, :], s2_t[0:64, cs])
                        nc.vector.tensor_mul(t2[64:128, :], ps[0:64, :], s2_t[64:128, cs])
                        nc.vector.tensor_add(dst[:, cs], t1[:], t2[:])

            # v: out tile [s tile 128, HG]
            for si in range(ST):
                ps = pp.tile([128, HG], f32, tag='pp', name='pp')
                for k in range(KT):
                    nc.tensor.matmul(
                        ps[:],
                        lhsT=xt[k][:, si * 128:(si + 1) * 128],
                        rhs=wts['v'][k][:],
                        start=(k == 0), stop=(k == KT - 1),
                    )
                nc.scalar.copy(vt[si][:], ps[:])

        # ---------------- Phase 2: attention ----------------
        if '2' not in PHASES:
            return nc
        attp = top.enter_context(tc.tile_pool(name='attp', bufs=1))
        att = [[attp.tile([128, 512], bf16, tag=f'att{h}_{c}', name=f'att{h}_{c}')
                for c in range(SC)] for h in range(G)]

        with tc.tile_pool(name='ps_p', bufs=int(os.environ.get('B_PS','4')), space='PSUM') as ps_p, \
             tc.tile_pool(name='pa_p', bufs=2, space='PSUM') as pa_p, \
             tc.tile_pool(name='pd_p', bufs=int(os.environ.get('B_PD','1')), space='PSUM') as pd_p, \
             tc.tile_pool(name='pb_p', bufs=1, space='PSUM') as pb_p, \
             tc.tile_pool(name='ex_p', bufs=int(os.environ.get('B_EX','3'))) as ex_p, \
             tc.tile_pool(name='nstg', bufs=int(os.environ.get('B_NS','4'))) as nstg:
            for h in range(G):
                hs_ = slice(h * 128, (h + 1) * 128)
                for c in range(SC):
                    cs = slice(c * 512, (c + 1) * 512)
                    pa = pa_p.tile([128, 512], f32, tag='pa', name='pa')
                    pd = pd_p.tile([1, 512], f32, tag='pd', name='pd')
                    for t in range(ST):
                        ps = ps_p.tile([128, 512], f32, tag='ps', name='ps')
                        nc.tensor.matmul(
                            ps[:],
                            lhsT=qk['k'][h][:, t * 128:(t + 1) * 128],
                            rhs=qk['q'][h][:, cs],
                            start=True, stop=True,
                        )
                        e = ex_p.tile([128, 512], bf16, tag='e', name='e')
                        nc.scalar.activation(e[:], ps[:], EXP,
                                             bias=mb_t[:, t:t + 1], scale=1.0)
                        nc.tensor.matmul(pa[:], lhsT=vt[t][:, hs_], rhs=e[:],
                                         start=(t == 0), stop=(t == ST - 1))
                        nc.tensor.matmul(pd[:], lhsT=ones_t[:], rhs=e[:],
                                         start=(t == 0), stop=(t == ST - 1))
                    rec = nstg.tile([1, 512], f32, tag='rec', name='rec')
                    nc.vector.reciprocal(rec[:], pd[:])
                    pb = pb_p.tile([128, 512], f32, tag='pb', name='pb')
                    nc.tensor.matmul(pb[:], lhsT=onesb_t[:], rhs=rec[:],
                                     start=True, stop=True)
                    sb = nstg.tile([128, 512], f32, tag='sb', name='sb')
                    nc.scalar.copy(sb[:], pb[:])
                    nc.vector.tensor_mul(att[h][c][:], pa[:], sb[:])

        # ---------------- Phase 3: o_proj ----------------
        if '3' not in PHASES:
            return nc
        with tc.tile_pool(name='wo_p', bufs=1) as wo_p, \
             tc.tile_pool(name='po_p', bufs=4, space='PSUM') as po_p, \
             tc.tile_pool(name='ostg', bufs=4) as ostg:
            wo_t = []
            for j in range(G):
                t = wo_p.tile([128, S], bf16, tag=f'wo{j}', name=f'wo{j}')
                nc.sync.dma_start(t[:], woT[j * 128:(j + 1) * 128, :])
                wo_t.append(t)
            for si in range(ST):
                cq, off = divmod(si, 4)
                off *= 128
                for nch in range(4):
                    ns = slice(nch * 512, (nch + 1) * 512)
                    po = po_p.tile([128, 512], f32, tag='po', name='po')
                    for h in range(G):
                        nc.tensor.matmul(
                            po[:],
                            lhsT=att[h][cq][:, off:off + 128],
                            rhs=wo_t[h][:, ns],
                            start=(h == 0), stop=(h == G - 1),
                        )
                    so = ostg.tile([128, 512], f32, tag='so', name='so')
                    nc.scalar.copy(so[:], po[:])
                    nc.sync.dma_start(out[si * 128:(si + 1) * 128, ns], so[:])
    nc.finalize()
    return nc


def _get_nc():
    if 'nc' not in _NC_CACHE:
        _NC_CACHE['nc'] = _build_nc()
    return _NC_CACHE['nc']


def _prep_in_maps(hidden_states, attention_mask, wq, wk, wv, wo):
    bf = ml_dtypes.bfloat16
    inv = 1.0 / (10000.0 ** (np.arange(0, HD, 2, dtype=np.float32) / np.float32(HD)))
    t = np.arange(S, dtype=np.float32)
    freqs = np.outer(t, inv).astype(np.float32)          # [S, 64]
    emb = np.concatenate([freqs, freqs], axis=1)         # [S, 128]
    cosT = np.ascontiguousarray(np.cos(emb).T.astype(np.float32))   # [128, S]
    sinT = np.ascontiguousarray(np.sin(emb).T.astype(np.float32))
    s2T = sinT.copy()
    s2T[:64] *= np.float32(-1.0)
    scale = np.float32(1.0) / np.sqrt(np.float32(HD))

    hs = np.asarray(hidden_states, dtype=np.float32)
    mask = np.asarray(attention_mask)
    wq = np.asarray(wq, dtype=np.float32)
    wk = np.asarray(wk, dtype=np.float32)
    wv = np.asarray(wv, dtype=np.float32)
    wo = np.asarray(wo, dtype=np.float32)

    in_maps = []
    for core in range(NCORES):
        b, g = divmod(core, G)
        cols = slice(g * HG, (g + 1) * HG)
        xTc = np.ascontiguousarray(hs[b].T).astype(bf)
        wqTc = np.ascontiguousarray((wq[cols, :] * scale).T).astype(bf)
        wkTc = np.ascontiguousarray(wk[cols, :].T).astype(bf)
        wvTc = np.ascontiguousarray(wv[cols, :].T).astype(bf)
        woTc = np.ascontiguousarray(wo[:, cols].T).astype(bf)
        mb = np.where(mask[b] == 0, np.float32(-1e30), np.float32(0.0))
        mbc = np.ascontiguousarray(mb.astype(np.float32).reshape(ST, 128).T)
        in_maps.append({
            'xT': xTc, 'wqT': wqTc, 'wkT': wkTc, 'wvT': wvTc, 'woT': woTc,
            'cosT': cosT, 's2T': s2T, 'maskb': mbc,
        })
    return in_maps


def kernel(hidden_states, attention_mask, wq, wk, wv, wo):
    _ensure_path()
    from concourse import bass_utils
    nc = _get_nc()
    in_maps = _prep_in_maps(hidden_states, attention_mask, wq, wk, wv, wo)
    res = bass_utils.run_bass_kernel_spmd(nc, in_maps, core_ids=list(range(NCORES)))
    outs = [r['out'] for r in res.results]
    full = np.empty((B, S, H), np.float32)
    for b in range(B):
        acc = outs[G * b].astype(np.float32)
        for g in range(1, G):
            acc = acc + outs[G * b + g]
        full[b] = acc
    return full


if __name__ == '__main__':
    rng = np.random.default_rng(0)
    ins = {
        'hidden_states': rng.standard_normal((B, S, H), dtype=np.float32),
        'attention_mask': np.ones((B, S), np.int32),
        'wq': rng.standard_normal((H, H), dtype=np.float32) / np.sqrt(H),
        'wk': rng.standard_normal((H, H), dtype=np.float32) / np.sqrt(H),
        'wv': rng.standard_normal((H, H), dtype=np.float32) / np.sqrt(H),
        'wo': rng.standard_normal((H, H), dtype=np.float32) / np.sqrt(H),
    }
    out = kernel(**ins)
    print('out', out.shape, out.dtype, float(np.abs(out).mean()))



# revision 26
# speedup vs baseline: 1.2592x; 1.2592x over previous
"""Llama attention layer on 8 TRN2 NeuronCores.

Sharding: core = (batch b in 0..1) x (head-group g in 0..3), 4 heads each.
Per core: full hidden_states[b] (transposed on host), column slices of
wq/wk/wv, row slice of wo.T. Host sums the 4 per-head-group o_proj
partials per batch.

v2 schedule (single fused pipeline, PE-bound ~330us):
  - head-0 q/k projections run k-outer over 8 concurrent PSUM groups so
    the PE ramps with the input-DMA drip instead of stalling on the
    first contraction.
  - v projections in two k-outer waves of 8 s-tiles.
  - attention blocks of [k=128, q=1024]; exp via ACT with mask bias;
    softmax denominator via DVE bf16 parity-chain adds over the 16 exp
    tiles + one Pool partition_all_reduce (no PE matmuls wasted on
    reductions); normalize with DVE reciprocal + mul from PSUM.
  - projections for head h+1 and the o_proj are interleaved into the
    attention loops as filler PE work so the PE never waits on ACT.
  - o_proj accumulates 4 heads in PSUM, ACT-copies to SBUF, DMAs out.
"""

import numpy as np
import ml_dtypes

B, S, H, NH, HD = 2, 2048, 2048, 16, 128
G = 4            # heads per core
HG = G * HD      # 512 head-dim columns per core
KT = H // 128    # 16 contraction chunks
ST = S // 128    # 16 sequence tiles of 128
QC = 1024        # attention q-chunk
NQC = S // QC    # 2 q-chunks per head
NCORES = 8
PA_LAG = 5       # emission lag (in t-iters) of pa matmuls behind scores

_NC_CACHE = {}


def _ensure_path():
    import sys
    for p in ('/opt/trn_rl_repo', '/opt/pypackages'):
        if p not in sys.path:
            sys.path.append(p)


def _build_nc():
    _ensure_path()
    from contextlib import ExitStack
    import concourse.tile as tile
    from concourse import bacc, mybir, bass_isa

    bf16 = mybir.dt.bfloat16
    f32 = mybir.dt.float32
    EXP = mybir.ActivationFunctionType.Exp
    RADD = bass_isa.ReduceOp.add

    nc = bacc.Bacc('TRN2', target_bir_lowering=False, debug=False)

    # host-packed layouts (few large DMAs; HWDGE costs ~630ns per DMA):
    # xH[kp, p, i*S+s]    = x.T[(2*kp+i)*128+p, s]
    # wqH[h, p, k*128+j]  = (wq_scaled).T[k*128+p, h*128+j]   (wkH same)
    # wvH[kc, p, r*512+j] = wv.T[(4*kc+r)*128+p, j]
    xH = nc.dram_tensor('xH', [KT // 2, 128, 2 * S], bf16, kind='ExternalInput')
    wqH = nc.dram_tensor('wqH', [G, 128, S], bf16, kind='ExternalInput')
    wkH = nc.dram_tensor('wkH', [G, 128, S], bf16, kind='ExternalInput')
    wvH = nc.dram_tensor('wvH', [KT // 4, 128, 4 * HG], bf16, kind='ExternalInput')
    woT = nc.dram_tensor('woT', [HG, H], bf16, kind='ExternalInput')
    cosT = nc.dram_tensor('cosT', [HD, S], f32, kind='ExternalInput')
    s2T = nc.dram_tensor('s2T', [HD, S], f32, kind='ExternalInput')
    maskb = nc.dram_tensor('maskb', [128, ST], f32, kind='ExternalInput')
    out = nc.dram_tensor('out', [S, H], bf16, kind='ExternalOutput')

    with tile.TileContext(nc) as tc, ExitStack() as top:
        persist = top.enter_context(tc.tile_pool(name='persist', bufs=1))

        cos_t = persist.tile([HD, S], f32, tag='cos', name='cos')
        s2_t = persist.tile([HD, S], f32, tag='s2', name='s2')
        mb_t = persist.tile([128, ST], f32, tag='mb', name='mb')

        # persistent activations
        vt = [persist.tile([128, HG], bf16, tag=f'v{i}', name=f'v{i}')
              for i in range(ST)]
        att = [[persist.tile([128, QC], bf16, tag=f'att{h}_{c}',
                             name=f'att{h}_{c}') for c in range(NQC)]
               for h in range(G)]

        qk_pool = top.enter_context(tc.tile_pool(name='qk', bufs=2))
        xw = top.enter_context(tc.tile_pool(name='xw', bufs=1))
        stg = top.enter_context(tc.tile_pool(name='stg', bufs=1))
        ep = top.enter_context(tc.tile_pool(name='ep', bufs=PA_LAG + 2))
        dnp = top.enter_context(tc.tile_pool(name='dnp', bufs=1))
        so_p = top.enter_context(tc.tile_pool(name='so_p', bufs=3))
        wqk_cm = tc.tile_pool(name='wqk', bufs=1)   # closed after h3 proj
        wqk = wqk_cm.__enter__()
        wvp_cm = tc.tile_pool(name='wvp', bufs=1)   # closed after v proj
        wvp = wvp_cm.__enter__()

        # -------- input DMAs: wv + x pairs drive the startup drip ---------
        # v projections run first (k-outer over x pairs as they land);
        # head-0 q/k weights arrive mid-drip, cos/s2 before the RoPE,
        # heads 1-3 weights well before their filler projections.
        wvh, xt2 = [], []
        wqh = {'q': [None] * G, 'k': [None] * G}
        for kp in range(KT // 2):
            if kp < KT // 4:
                w = wvp.tile([128, 4 * HG], bf16, tag=f'wv{kp}', name=f'wv{kp}')
                nc.sync.dma_start(w[:], wvH[kp])
                wvh.append(w)
            t = xw.tile([128, 2 * S], bf16, tag=f'x{kp}', name=f'x{kp}')
            nc.sync.dma_start(t[:], xH[kp])
            xt2.append(t)
            if kp == 0:
                for nm in ('q', 'k'):
                    w = wqk.tile([128, S], bf16, tag=f'w{nm}0', name=f'w{nm}0')
                    nc.sync.dma_start(w[:], (wqH if nm == 'q' else wkH)[0])
                    wqh[nm][0] = w
        nc.sync.dma_start(cos_t[:], cosT[:])
        nc.sync.dma_start(s2_t[:], s2T[:])
        nc.sync.dma_start(mb_t[:], maskb[:])
        for h in range(1, G):
            for nm in ('q', 'k'):
                w = wqk.tile([128, S], bf16, tag=f'w{nm}{h}', name=f'w{nm}{h}')
                nc.sync.dma_start(w[:], (wqH if nm == 'q' else wkH)[h])
                wqh[nm][h] = w

        def x_ap(k, cols):
            # columns `cols` of packed x chunk k
            base = (k & 1) * S
            return xt2[k >> 1][:, base + cols.start:base + cols.stop]

        def wv_ap(k):
            r = k & 3
            return wvh[k >> 2][:, r * HG:(r + 1) * HG]

        qk = {'q': [None] * G, 'k': [None] * G}

        def rope(dst_tile, cs, ps):
            # dst[:, cs] = ps*cos + rot(ps)*s2  (rot: [d] <- [(d+64)%128])
            nc.vector.tensor_mul(dst_tile[:, cs], ps[:], cos_t[:, cs])
            t2 = stg.tile([128, 512], bf16, tag='t2', name='t2', bufs=3)
            nc.vector.tensor_mul(t2[0:64, :], ps[64:128, :], s2_t[0:64, cs])
            nc.vector.tensor_mul(t2[64:128, :], ps[0:64, :], s2_t[64:128, cs])
            nc.vector.tensor_add(dst_tile[:, cs], dst_tile[:, cs], t2[:])

        # ----- startup compute: v waves first, then head-0 q/k, ropes ----
        # A dummy matmul stream on the spare PSUM bank keeps the PE
        # p-state ramp hot through the DMA drip (idle gaps would drop
        # every following matmul to the mid/low-speed p-state).
        dum_sb = persist.tile([128, 256], bf16, tag='dum', name='dum')
        nc.vector.memset(dum_sb[:], 0.0)
        with tc.tile_pool(name='pp0', bufs=1, space='PSUM') as pp0:
            # bank order matters: banks are reused by the attention pools
            # in first-fit order, so allocate in rope-priority order and
            # keep the dummy/last-rope bank highest.
            wA = {si: pp0.tile([128, HG], f32, tag=f'g{si}', name=f'vps{si}')
                  for si in range(7)}
            dum_ps = pp0.tile([128, 512], f32, tag='dum', name='dum_ps')

            def dummies(n):
                for i in range(n):
                    j = i & 3
                    nc.tensor.matmul(
                        dum_ps[:, j * 128:(j + 1) * 128],
                        lhsT=dum_sb[:, 0:128], rhs=dum_sb[:, 128:256],
                        start=True, stop=True)

            def v_tile_group(si, pool, tag):
                w_ps = pool.tile([128, HG], f32, tag=tag, name=f'vps{si}')
                for k in range(KT):
                    nc.tensor.matmul(
                        w_ps[:],
                        lhsT=x_ap(k, slice(si * 128, (si + 1) * 128)),
                        rhs=wv_ap(k),
                        start=(k == 0), stop=(k == KT - 1),
                    )
                if si & 1:
                    nc.vector.tensor_copy(vt[si][:], w_ps[:])
                else:
                    nc.scalar.copy(vt[si][:], w_ps[:])

            dummies(48)
            # wave A: si 0..6 k-outer, paced by the x-pair drip
            for k in range(KT):
                for si in range(7):
                    nc.tensor.matmul(
                        wA[si][:],
                        lhsT=x_ap(k, slice(si * 128, (si + 1) * 128)),
                        rhs=wv_ap(k),
                        start=(k == 0), stop=(k == KT - 1),
                    )
                dummies(3)
            for si in range(7):
                if si & 1:
                    nc.vector.tensor_copy(vt[si][:], wA[si][:])
                else:
                    nc.scalar.copy(vt[si][:], wA[si][:])

            # head-0 q/k projections: k-outer, 8 groups, full speed.
            # Rope order == bank order; wave B then chases the rope chain
            # bank-by-bank so the 18us of serial DVE rope overlaps PE.
            for nm in ('q', 'k'):
                qk[nm][0] = qk_pool.tile([128, S], bf16, tag=nm, name=f'{nm}0')
            groups = [('q', 0), ('q', 1), ('k', 0), ('k', 1),
                      ('k', 2), ('k', 3), ('q', 2), ('q', 3)]
            tags = [f'g{j}' for j in range(7)] + ['dum']
            g_ps = {gc: pp0.tile([128, 512], f32, tag=tags[j], name=f'qk{j}')
                    for j, gc in enumerate(groups)}
            for k in range(KT):
                for nm, c in groups:
                    nc.tensor.matmul(
                        g_ps[(nm, c)][:],
                        lhsT=wqh[nm][0][:, k * 128:(k + 1) * 128],
                        rhs=x_ap(k, slice(c * 512, (c + 1) * 512)),
                        start=(k == 0), stop=(k == KT - 1),
                    )
            for nm, c in groups:
                rope(qk[nm][0], slice(c * 512, (c + 1) * 512), g_ps[(nm, c)])
            # wave B + v14/15: PE work that overlaps the rope chain
            for si in range(7, 14):
                v_tile_group(si, pp0, f'g{si - 7}')
            v_tile_group(14, pp0, 'g0')
            v_tile_group(15, pp0, 'g1')
        wvp_cm.__exit__(None, None, None)
        wo_t = []   # filled at the h==G-2 pool swap

        # ------------- filler stream: projections for heads 1..3 ----------
        pa_cm = tc.tile_pool(name='pa_p', bufs=1, space='PSUM')
        pa_p = pa_cm.__enter__()
        ps_cm = tc.tile_pool(name='ps_p', bufs=2, space='PSUM')
        ps_p = ps_cm.__enter__()
        pp_cm = tc.tile_pool(name='pp', bufs=2, space='PSUM')
        pp = pp_cm.__enter__()
        filler = []

        def make_proj_group(nm, h, c):
            state = {}
            cs = slice(c * 512, (c + 1) * 512)

            def unit(k0, state=state, nm=nm, h=h, c=c, cs=cs):
                if k0 == 0:
                    if qk[nm][h] is None:
                        qk[nm][h] = qk_pool.tile([128, S], bf16, tag=nm,
                                                 name=f'{nm}{h}')
                    state['ps'] = pp.tile([128, 512], f32, tag='pp', name='pp')
                ps = state['ps']
                for k in range(k0, k0 + 4):
                    nc.tensor.matmul(
                        ps[:],
                        lhsT=wqh[nm][h][:, k * 128:(k + 1) * 128],
                        rhs=x_ap(k, cs),
                        start=(k == 0), stop=(k == KT - 1),
                    )
                if k0 == 12:
                    rope(qk[nm][h], cs, ps)
            return [lambda k0=k0: unit(k0) for k0 in (0, 4, 8, 12)]

        for h in range(1, G):
            for nm in ('q', 'k'):
                for c in range(4):
                    filler.extend(make_proj_group(nm, h, c))

        fill_i = 0

        def emit_fillers(n):
            nonlocal fill_i
            end = min(fill_i + n, len(filler))
            while fill_i < end:
                filler[fill_i]()
                fill_i += 1

        # ---------------- attention ----------------
        po_p = None

        def o_proj_group(si, nch):
            qc, r = divmod(si, ST // NQC)
            ns = slice(nch * 512, (nch + 1) * 512)
            po = po_p.tile([128, 512], f32, tag='po', name='po')
            for h in range(G):
                nc.tensor.matmul(
                    po[:],
                    lhsT=att[h][qc][:, r * 128:(r + 1) * 128],
                    rhs=wo_t[h][:, ns],
                    start=(h == 0), stop=(h == G - 1),
                )
            so = so_p.tile([128, 512], bf16, tag='so', name='so')
            if (si + nch) & 1:
                nc.scalar.copy(so[:], po[:])
            else:
                nc.vector.tensor_copy(so[:], po[:])
            nc.sync.dma_start(out[si * 128:(si + 1) * 128, ns], so[:])

        for h in range(G):
            hs_ = slice(h * 128, (h + 1) * 128)
            for c in range(NQC):
                if h == G - 1 and c == 1:
                    # o_proj for q-chunk 0 interleaves into this block;
                    # start late enough that att[3][0] (end of previous
                    # block) is ready before the first group hits the PE
                    # queue; si==7 is held back to cover the final
                    # normalize-chain latency after the last scores.
                    filler[:] = [
                        (lambda si=si, nch=nch: o_proj_group(si, nch))
                        for si in range(ST // NQC - 1)
                        for nch in range(4)
                    ]
                    fill_i = 0
                pa = pa_p.tile([128, QC], f32, tag='pa', name='pa')
                acc = {}
                es = [None] * ST

                def emit_pa(t):
                    for half in range(2):
                        fs = slice(half * 512, (half + 1) * 512)
                        nc.tensor.matmul(
                            pa[:, fs], lhsT=vt[t][:, hs_], rhs=es[t][:, fs],
                            start=(t == 0), stop=(t == ST - 1),
                        )

                for t in range(ST):
                    ps = ps_p.tile([128, QC], f32, tag='ps', name='ps')
                    for half in range(2):
                        fs = slice(half * 512, (half + 1) * 512)
                        nc.tensor.matmul(
                            ps[:, fs],
                            lhsT=qk['k'][h][:, t * 128:(t + 1) * 128],
                            rhs=qk['q'][h][:, c * QC + half * 512:
                                           c * QC + (half + 1) * 512],
                            start=True, stop=True,
                        )
                    e = ep.tile([128, QC], bf16, tag='e', name='e')
                    nc.scalar.activation(e[:], ps[:], EXP,
                                         bias=mb_t[:, t:t + 1], scale=1.0)
                    es[t] = e
                    if t >= PA_LAG:
                        emit_pa(t - PA_LAG)
                    # denominator parity chains on DVE (bf16)
                    par = t & 1
                    if t >= 2:
                        if t < 4:
                            a = dnp.tile([128, QC], bf16, tag=f'acc{par}',
                                         name=f'acc{par}')
                            acc[par] = a
                            nc.vector.tensor_add(a[:], es[t - 2][:], e[:])
                        else:
                            nc.vector.tensor_add(acc[par][:], acc[par][:],
                                                 e[:])
                    if h == G - 1:
                        if c == 1 and t >= 6:
                            emit_fillers(3)
                    elif h == 0 and c == 1:
                        emit_fillers(2)   # head-1 proj must finish here
                    elif h > 0:
                        emit_fillers(1)
                for t in range(ST - PA_LAG, ST):
                    emit_pa(t)
                ar = dnp.tile([128, QC], f32, tag='ar', name='ar')
                for half in range(2):
                    fs = slice(half * 512, (half + 1) * 512)
                    nc.vector.tensor_add(acc[0][:, fs], acc[0][:, fs],
                                         acc[1][:, fs])
                    nc.gpsimd.partition_all_reduce(ar[:, fs], acc[0][:, fs],
                                                   128, RADD)
                    nc.vector.reciprocal(ar[:, fs], ar[:, fs])
                    nc.vector.tensor_mul(att[h][c][:, fs], pa[:, fs],
                                         ar[:, fs])

            if h == G - 2:
                # last projections done: drop wq/wk + proj PSUM, bring in
                # wo + o_proj PSUM (fresh ps/pa pools for head 3)
                emit_fillers(len(filler))
                pp_cm.__exit__(None, None, None)
                ps_cm.__exit__(None, None, None)
                pa_cm.__exit__(None, None, None)
                wqk_cm.__exit__(None, None, None)
                wop_cm = tc.tile_pool(name='wop', bufs=1)
                wop = wop_cm.__enter__()
                for j in range(G):
                    w = wop.tile([128, S], bf16, tag=f'wo{j}', name=f'wo{j}')
                    nc.sync.dma_start(w[:], woT[j * 128:(j + 1) * 128, :])
                    wo_t.append(w)
                pa_cm = tc.tile_pool(name='pa_p', bufs=1, space='PSUM')
                pa_p = pa_cm.__enter__()
                ps_cm = tc.tile_pool(name='ps_p', bufs=2, space='PSUM')
                ps_p = ps_cm.__enter__()
                po_cm = tc.tile_pool(name='po_p', bufs=2, space='PSUM')
                po_p = po_cm.__enter__()
                filler[:] = []
                fill_i = 0

        # ---------------- o_proj tail ----------------
        emit_fillers(len(filler))
        for nch in range(4):
            o_proj_group(ST // NQC - 1, nch)   # si 7: att[3][0], ready early
        for si in range(ST // NQC, ST):
            for nch in range(4):
                o_proj_group(si, nch)
        po_cm.__exit__(None, None, None)
        ps_cm.__exit__(None, None, None)
        pa_cm.__exit__(None, None, None)
        wop_cm.__exit__(None, None, None)
    nc.finalize()
    return nc


def _get_nc():
    if 'nc' not in _NC_CACHE:
        _NC_CACHE['nc'] = _build_nc()
    return _NC_CACHE['nc']


def _prep_in_maps(hidden_states, attention_mask, wq, wk, wv, wo):
    bf = ml_dtypes.bfloat16
    inv = 1.0 / (10000.0 ** (np.arange(0, HD, 2, dtype=np.float32) / np.float32(HD)))
    t = np.arange(S, dtype=np.float32)
    freqs = np.outer(t, inv).astype(np.float32)          # [S, 64]
    emb = np.concatenate([freqs, freqs], axis=1)         # [S, 128]
    cosT = np.ascontiguousarray(np.cos(emb).T.astype(np.float32))   # [128, S]
    sinT = np.ascontiguousarray(np.sin(emb).T.astype(np.float32))
    s2T = sinT.copy()
    s2T[:64] *= np.float32(-1.0)
    scale = np.float32(1.0) / np.sqrt(np.float32(HD))

    hs = np.asarray(hidden_states, dtype=np.float32)
    mask = np.asarray(attention_mask)
    wq = np.asarray(wq, dtype=np.float32)
    wk = np.asarray(wk, dtype=np.float32)
    wv = np.asarray(wv, dtype=np.float32)
    wo = np.asarray(wo, dtype=np.float32)

    def pack_x(xT):           # [H,S] -> [KT//2, 128, 2S]
        return np.ascontiguousarray(
            xT.reshape(KT // 2, 2, 128, S).transpose(0, 2, 1, 3)
            .reshape(KT // 2, 128, 2 * S))

    def pack_w(wT):           # [H,HG] -> [G, 128, S] (per-head k-major)
        return np.ascontiguousarray(
            wT.reshape(KT, 128, G, 128).transpose(2, 1, 0, 3)
            .reshape(G, 128, KT * 128))

    def pack_wv(wT):          # [H,HG] -> [KT//4, 128, 4*HG]
        return np.ascontiguousarray(
            wT.reshape(KT // 4, 4, 128, HG).transpose(0, 2, 1, 3)
            .reshape(KT // 4, 128, 4 * HG))

    in_maps = []
    for core in range(NCORES):
        b, g = divmod(core, G)
        cols = slice(g * HG, (g + 1) * HG)
        xTc = hs[b].T.astype(bf)
        wqTc = (wq[cols, :] * scale).T.astype(bf)
        wkTc = wk[cols, :].T.astype(bf)
        wvTc = wv[cols, :].T.astype(bf)
        woTc = np.ascontiguousarray(wo[:, cols].T).astype(bf)
        mb = np.where(mask[b] == 0, np.float32(-1e30), np.float32(0.0))
        mbc = np.ascontiguousarray(mb.astype(np.float32).reshape(ST, 128).T)
        in_maps.append({
            'xH': pack_x(xTc), 'wqH': pack_w(wqTc), 'wkH': pack_w(wkTc),
            'wvH': pack_wv(wvTc), 'woT': woTc,
            'cosT': cosT, 's2T': s2T, 'maskb': mbc,
        })
    return in_maps


def kernel(hidden_states, attention_mask, wq, wk, wv, wo):
    _ensure_path()
    from concourse import bass_utils
    nc = _get_nc()
    in_maps = _prep_in_maps(hidden_states, attention_mask, wq, wk, wv, wo)
    res = bass_utils.run_bass_kernel_spmd(nc, in_maps, core_ids=list(range(NCORES)))
    outs = [r['out'] for r in res.results]
    full = np.empty((B, S, H), np.float32)
    for b in range(B):
        acc = outs[G * b].astype(np.float32)
        for g in range(1, G):
            acc = acc + outs[G * b + g]
        full[b] = acc
    return full


if __name__ == '__main__':
    rng = np.random.default_rng(0)
    ins = {
        'hidden_states': rng.standard_normal((B, S, H), dtype=np.float32),
        'attention_mask': np.ones((B, S), np.int32),
        'wq': rng.standard_normal((H, H), dtype=np.float32) / np.sqrt(H),
        'wk': rng.standard_normal((H, H), dtype=np.float32) / np.sqrt(H),
        'wv': rng.standard_normal((H, H), dtype=np.float32) / np.sqrt(H),
        'wo': rng.standard_normal((H, H), dtype=np.float32) / np.sqrt(H),
    }
    out = kernel(**ins)
    print('out', out.shape, out.dtype, float(np.abs(out).mean()))
